# revision 1
# baseline (speedup 1.0000x reference)
"""Fused per-pixel kernel for nn_KernelFusion_19026705121450 on 8 trn2 cores.

Math: the reference computes, per pixel q = z[b,:,h,w] (3 channels):
    z_map = Wz q + bz                      (64-dim)
    t_b   = Wt text_b + bt                 (64-dim, per batch)
    dist  = ||z_map - t_b||^2
    k_lin = z_map . t_b
    k     = (w0 exp(-g*dist) + w1 k_lin + w2 (a k_lin + c)^2) / (sum w + 1e-8)
    out   = (1 + sigmoid(k)) * z_map -> 1x1 conv Wo + bo

Every 64-dim reduction is a quadratic/linear form in the 3-dim q, so on
host (fp64) we collapse:
    dist  = ||L^T q + r_b||^2 + rho_b      (L = chol(Wz^T Wz), 3x3)
    k_lin = u_b . q + s_b
    out_o = (1+sigmoid(k)) * (M q + m)_o + bo_o,  M = Wo Wz (3x3)

Device kernel is ~34 elementwise ops per tile; no matmuls, no HID dim.
Sharding: pure data parallel, 2 batches/core. On-chip layout [128, 1024]:
partition p -> (batch p//64, row p%64), free f -> pixel (p%64)*1024 + f.
Per-batch constants ride as [128,1] per-partition AP operands, so the
compiled program is input-independent (cached across calls).
"""

import sys

if "/opt/trn_rl_repo" not in sys.path:
    sys.path.insert(0, "/opt/trn_rl_repo")

import numpy as np

import concourse.bass as bass
import concourse.bacc as bacc
import concourse.mybir as mybir
from concourse.tile import TileContext
from concourse import bass_utils

F32 = mybir.dt.float32
AF = mybir.ActivationFunctionType
OP = mybir.AluOpType

NCORES = 8
BPC = 2          # batches per core
ROWS = 64        # partition rows per batch
P = 128          # partitions
FREE = 1024      # free dim (ROWS * FREE = H*W)
NCONST = 33

# const column indices
C_R0, C_R1, C_R2, C_BETA0, C_NEGG = 0, 1, 2, 3, 4
C_U0, C_U1, C_U2, C_S = 5, 6, 7, 8
C_L00, C_L10, C_L20, C_L11, C_L21, C_L22 = 9, 10, 11, 12, 13, 14
C_APOLY, C_BPOLY, C_W1P = 15, 16, 17
C_M = 18          # 18..26 row-major M[o][c]
C_MV = 27         # 27..29 m
C_BO = 30         # 30..32 out bias

_NC_CACHE: dict = {}


def _build_nc(sw0_pos: bool, sw2_pos: bool, nchunk: int = 2,
              use_gpsimd: bool = True, cfg: dict | None = None):
    # cfg knobs: d_eng, to_engs (3-tuple), oo_engs (3-tuple), g1_eng,
    # start_kla, start_e1  ('act'|'dve'|'pool')
    cfg = dict(cfg or {})
    d_eng = cfg.get("d_eng", "pool" if use_gpsimd else "dve")
    to_engs = cfg.get("to_engs", ("pool" if use_gpsimd else "dve",) * 3)
    oo_engs = cfg.get("oo_engs", ("act",) * 3)
    g1_eng = cfg.get("g1_eng", "act")
    start_kla = cfg.get("start_kla", "act")
    start_e1 = cfg.get("start_e1", "act")
    start_ya = cfg.get("start_ya", ("act",) * 3)
    inplace = cfg.get("inplace", False)
    wbufs_cfg = cfg.get("bufs", None)
    out_dma = cfg.get("out_dma", "sync")
    nc = bacc.Bacc("TRN2", target_bir_lowering=False)
    # packed input: [consts (NCONST) | chunk0: z0|z1|z2 | chunk1: z0|z1|z2]
    # one DMA per chunk -> one wait semaphore per chunk (walrus rejects
    # instructions with too many sync waits).
    cw = FREE // nchunk
    zc_cols = NCONST + 3 * FREE
    zc = nc.dram_tensor("zc", [P, zc_cols], F32, kind="ExternalInput")
    out = nc.dram_tensor("out_shard", [3, P, FREE], F32, kind="ExternalOutput")

    op_k2 = OP.add if sw2_pos else OP.subtract
    op_k1 = OP.add if sw0_pos else OP.subtract
    cf = FREE // nchunk

    def E(which):
        return {"act": nc.scalar, "dve": nc.vector, "pool": nc.gpsimd}[which]

    with TileContext(nc) as tc:
        with tc.tile_pool(name="cpool", bufs=1) as cpool, \
             tc.tile_pool(name="work", bufs=1) as pool:
            zt = cpool.tile([P, zc_cols], F32, name="zt")
            # chunk 0 DMA carries the consts columns too
            nc.sync.dma_start(out=zt[:, 0:NCONST + 3 * cw],
                              in_=zc[:, 0:NCONST + 3 * cw])
            for ci in range(1, nchunk):
                a = NCONST + 3 * cw * ci
                nc.sync.dma_start(out=zt[:, a:a + 3 * cw],
                                  in_=zc[:, a:a + 3 * cw])

            def col(j):
                return zt[:, j:j + 1]

            for ci in range(nchunk):
                fs = ci * cf
                sl = (slice(None), slice(fs, fs + cf))
                base = NCONST + 3 * cw * ci
                z0 = zt[:, base:base + cw]
                z1 = zt[:, base + cw:base + 2 * cw]
                z2 = zt[:, base + 2 * cw:base + 3 * cw]

                def t(tag):
                    return pool.tile([P, cf], F32, tag=f"{tag}_{ci}",
                                     name=f"{tag}_{ci}")

                # dist path: e = L^T q + r, dist = sum e_i^2 + rho
                e0 = t("e0")
                nc.scalar.activation(e0[:, :], z0[:, :], AF.Identity,
                                     bias=col(C_R0), scale=col(C_L00))
                e0b = t("e0b")
                nc.vector.scalar_tensor_tensor(e0b[:, :], z1[:, :], col(C_L10),
                                               e0[:, :], OP.mult, OP.add)
                e0c = e0b if inplace else t("e0c")
                nc.vector.scalar_tensor_tensor(e0c[:, :], z2[:, :], col(C_L20),
                                               e0b[:, :], OP.mult, OP.add)
                sq0 = t("sq0")
                nc.scalar.activation(sq0[:, :], e0c[:, :], AF.Square)

                e1 = t("e1")
                if start_e1 == "act":
                    nc.scalar.activation(e1[:, :], z1[:, :], AF.Identity,
                                         bias=col(C_R1), scale=col(C_L11))
                else:
                    E(start_e1).tensor_scalar(e1[:, :], z1[:, :], col(C_L11),
                                              col(C_R1), OP.mult, OP.add)
                e1b = t("e1b")
                nc.vector.scalar_tensor_tensor(e1b[:, :], z2[:, :], col(C_L21),
                                               e1[:, :], OP.mult, OP.add)
                sq1 = t("sq1")
                nc.scalar.activation(sq1[:, :], e1b[:, :], AF.Square)

                sq2 = t("sq2")
                nc.scalar.activation(sq2[:, :], z2[:, :], AF.Square,
                                     bias=col(C_R2), scale=col(C_L22))

                d1 = t("d1")
                d2 = t("d2")
                E(d_eng).tensor_add(out=d1[:, :], in0=sq0[:, :], in1=sq1[:, :])
                E(d_eng).tensor_add(out=d2[:, :], in0=d1[:, :], in1=sq2[:, :])

                # krbf = |w0'| * exp(-g*dist) via bias fold
                krbf = t("krbf")
                nc.scalar.activation(krbf[:, :], d2[:, :], AF.Exp,
                                     bias=col(C_BETA0), scale=col(C_NEGG))

                # k_lin = u . q + s
                kla = t("kla")
                if start_kla == "act":
                    nc.scalar.activation(kla[:, :], z0[:, :], AF.Identity,
                                         bias=col(C_S), scale=col(C_U0))
                else:
                    E(start_kla).tensor_scalar(kla[:, :], z0[:, :], col(C_U0),
                                               col(C_S), OP.mult, OP.add)
                klb = t("klb")
                nc.vector.scalar_tensor_tensor(klb[:, :], z1[:, :], col(C_U1),
                                               kla[:, :], OP.mult, OP.add)
                kl = klb if inplace else t("kl")
                nc.vector.scalar_tensor_tensor(kl[:, :], z2[:, :], col(C_U2),
                                               klb[:, :], OP.mult, OP.add)

                # p2 = |w2'| (a k_lin + c)^2
                p2 = t("p2")
                nc.scalar.activation(p2[:, :], kl[:, :], AF.Square,
                                     bias=col(C_BPOLY), scale=col(C_APOLY))

                kb = t("kb")
                nc.vector.scalar_tensor_tensor(kb[:, :], kl[:, :], col(C_W1P),
                                               p2[:, :], OP.mult, op_k2)
                kt = t("kt")
                if op_k1 == OP.add:
                    nc.vector.tensor_add(out=kt[:, :], in0=kb[:, :], in1=krbf[:, :])
                else:
                    nc.vector.tensor_sub(out=kt[:, :], in0=kb[:, :], in1=krbf[:, :])

                sig = t("sig")
                nc.scalar.activation(sig[:, :], kt[:, :], AF.Sigmoid)
                g1 = t("g1")
                if g1_eng == "act":
                    nc.scalar.activation(g1[:, :], sig[:, :], AF.Identity,
                                         bias=1.0)
                else:
                    E(g1_eng).tensor_scalar_add(g1[:, :], sig[:, :], 1.0)

                # y_o = M q + m ; out_o = g1*y_o + bo
                for o in range(3):
                    ya = t(f"ya{o}")
                    if start_ya[o] == "act":
                        nc.scalar.activation(ya[:, :], z0[:, :], AF.Identity,
                                             bias=col(C_MV + o),
                                             scale=col(C_M + 3 * o))
                    else:
                        E(start_ya[o]).tensor_scalar(ya[:, :], z0[:, :],
                                                     col(C_M + 3 * o),
                                                     col(C_MV + o),
                                                     OP.mult, OP.add)
                    yb = t(f"yb{o}")
                    nc.vector.scalar_tensor_tensor(yb[:, :], z1[:, :],
                                                   col(C_M + 3 * o + 1),
                                                   ya[:, :], OP.mult, OP.add)
                    yc = yb if inplace else t(f"yc{o}")
                    nc.vector.scalar_tensor_tensor(yc[:, :], z2[:, :],
                                                   col(C_M + 3 * o + 2),
                                                   yb[:, :], OP.mult, OP.add)
                    to = t(f"to{o}")
                    E(to_engs[o]).tensor_mul(out=to[:, :], in0=yc[:, :],
                                             in1=g1[:, :])
                    oo = to if inplace else t(f"oo{o}")
                    if oo_engs[o] == "act":
                        nc.scalar.activation(oo[:, :], to[:, :], AF.Identity,
                                             bias=col(C_BO + o))
                    else:
                        E(oo_engs[o]).tensor_scalar_add(oo[:, :], to[:, :],
                                                        col(C_BO + o))
                    dma_eng = nc.scalar if out_dma == "scalar" else nc.sync
                    dma_eng.dma_start(out=out[o][sl], in_=oo[:, :])
    nc.compile()
    return nc


def _get_nc(sw0_pos, sw2_pos, nchunk=2, use_gpsimd=True, cfg=None):
    key = (sw0_pos, sw2_pos, nchunk, use_gpsimd,
           tuple(sorted((cfg or {}).items())))
    if key not in _NC_CACHE:
        _NC_CACHE[key] = _build_nc(sw0_pos, sw2_pos, nchunk, use_gpsimd, cfg)
    return _NC_CACHE[key]


def _host_prep(inputs, nchunk=2):
    d = {k: np.asarray(v, dtype=np.float64) for k, v in inputs.items()}
    z = np.ascontiguousarray(np.asarray(inputs["z"], dtype=np.float32))
    B, C, H, W = z.shape
    HW = H * W
    Wz, bz = d["z_proj_w"], d["z_proj_b"]
    Wt, bt = d["text_proj_w"], d["text_proj_b"]
    Wo, bo = d["out_w"], d["out_b"]
    gamma = np.exp(d["log_gamma"])
    alpha, c, w = d["alpha"], d["c"], d["w"]
    sumw = w.sum() + 1e-8
    w0p, w1p, w2p = w[0] / sumw, w[1] / sumw, w[2] / sumw

    t = d["text_vec"] @ Wt.T + bt                       # [B, HID]
    G = Wz.T @ Wz                                       # [3,3]
    L = np.linalg.cholesky(G)                           # may raise -> caller
    delta = bz[None, :] - t                             # [B, HID]
    v = delta @ Wz                                      # [B, 3]
    cdist = (delta ** 2).sum(1)                         # [B]
    r = np.linalg.solve(L, v.T).T                       # [B, 3], L r = v
    rho = cdist - (r ** 2).sum(1)
    u = t @ Wz                                          # [B, 3]
    s = t @ bz                                          # [B]
    if w0p == 0.0:
        beta0 = np.full(B, -1e30)
    else:
        beta0 = -gamma * rho + np.log(abs(w0p))
    aPoly = alpha * np.sqrt(abs(w2p))
    bPoly = c * np.sqrt(abs(w2p))
    M = Wo @ Wz                                         # [3,3]
    m = Wo @ bz                                         # [3]

    cb = np.zeros((B, NCONST), dtype=np.float64)
    cb[:, C_R0], cb[:, C_R1], cb[:, C_R2] = r[:, 0], r[:, 1], r[:, 2]
    cb[:, C_BETA0] = beta0
    cb[:, C_NEGG] = -gamma
    cb[:, C_U0], cb[:, C_U1], cb[:, C_U2] = u[:, 0], u[:, 1], u[:, 2]
    cb[:, C_S] = s
    cb[:, C_L00], cb[:, C_L10], cb[:, C_L20] = L[0, 0], L[1, 0], L[2, 0]
    cb[:, C_L11], cb[:, C_L21], cb[:, C_L22] = L[1, 1], L[2, 1], L[2, 2]
    cb[:, C_APOLY], cb[:, C_BPOLY], cb[:, C_W1P] = aPoly, bPoly, w1p
    for o in range(3):
        cb[:, C_M + 3 * o: C_M + 3 * o + 3] = M[o]
        cb[:, C_MV + o] = m[o]
        cb[:, C_BO + o] = bo[o]
    cb = cb.astype(np.float32)

    cw = FREE // nchunk
    in_maps = []
    for core in range(NCORES):
        zs = np.empty((3, P, FREE), dtype=np.float32)
        cs = np.empty((P, NCONST), dtype=np.float32)
        for j in range(BPC):
            b = core * BPC + j
            zs[:, j * ROWS:(j + 1) * ROWS, :] = z[b].reshape(3, ROWS, FREE)
            cs[j * ROWS:(j + 1) * ROWS, :] = cb[b]
        packed = np.empty((P, NCONST + 3 * FREE), dtype=np.float32)
        packed[:, :NCONST] = cs
        for ci in range(nchunk):
            base = NCONST + 3 * cw * ci
            for c in range(3):
                packed[:, base + c * cw:base + (c + 1) * cw] = \
                    zs[c, :, ci * cw:(ci + 1) * cw]
        in_maps.append({"zc": packed})
    return in_maps, (w0p >= 0.0, w2p >= 0.0), (B, C, H, W)


def _numpy_fallback(inputs):
    d = {k: np.asarray(v, dtype=np.float64) for k, v in inputs.items()}
    z, Wz, bz = d["z"], d["z_proj_w"], d["z_proj_b"]
    t = d["text_vec"] @ d["text_proj_w"].T + d["text_proj_b"]
    zm = np.einsum("bchw,oc->bohw", z, Wz) + bz[None, :, None, None]
    gamma = np.exp(d["log_gamma"])
    diff = zm - t[:, :, None, None]
    dist = (diff * diff).sum(1)
    klin = np.einsum("bchw,bc->bhw", zm, t)
    krbf = np.exp(-gamma * dist)
    kpoly = (d["alpha"] * klin + d["c"]) ** 2
    w = d["w"]
    k = (w[0] * krbf + w[1] * klin + w[2] * kpoly) / (w.sum() + 1e-8)
    zf = zm * (1.0 + 1.0 / (1.0 + np.exp(-k[:, None])))
    out = np.einsum("bchw,oc->bohw", zf, d["out_w"]) + d["out_b"][None, :, None, None]
    return out.astype(np.float32)


BEST_CFG: dict = {"d_eng": "dve", "to_engs": ("dve", "dve", "dve")}
BEST_NCHUNK = 2
BEST_GPSIMD = False


def run(inputs, trace=False, nchunk=None, use_gpsimd=None, cfg=None):
    if nchunk is None:
        nchunk = BEST_NCHUNK
    if use_gpsimd is None:
        use_gpsimd = BEST_GPSIMD
    if cfg is None:
        cfg = BEST_CFG
    try:
        in_maps, (sw0, sw2), (B, C, H, W) = _host_prep(inputs, nchunk)
    except np.linalg.LinAlgError:
        return _numpy_fallback(inputs), None
    nc = _get_nc(sw0, sw2, nchunk, use_gpsimd, cfg)
    res = bass_utils.run_bass_kernel_spmd(
        nc, in_maps, core_ids=list(range(NCORES)), trace=trace)
    out = np.empty((B, C, H, W), dtype=np.float32)
    for core in range(NCORES):
        o = res.results[core]["out_shard"]          # [3, P, FREE]
        for j in range(BPC):
            b = core * BPC + j
            out[b] = o[:, j * ROWS:(j + 1) * ROWS, :].reshape(C, H, W)
    return out, res


def kernel(**inputs):
    out, _ = run(inputs, trace=False)
    return out



# revision 3
# speedup vs baseline: 1.4102x; 1.4102x over previous
"""Fused per-pixel kernel for nn_KernelFusion_19026705121450 on 8 trn2 cores.

Math (per pixel q = z[b,:,h,w], 3 channels):
    z_map = Wz q + bz; t_b = Wt text_b + bt
    dist  = ||z_map - t_b||^2 ; k_lin = z_map . t_b
    k     = (w0 exp(-g dist) + w1 k_lin + w2 (a k_lin + c)^2) / (sum w + eps)
    out   = (1 + sigmoid(k)) * z_map -> 1x1 conv Wo + bo

All HID=64 reductions collapse to quadratic/affine forms of q (host, fp64):
    dist  = e0^2+e1^2+e2^2 + rho,  e = L^T q + r  (L = chol(Wz^T Wz))
    k_lin = u.q + s ;  y_o = M_o.q + m_o  (M = Wo Wz)
Device streams are fp16; shifted basis z''_c = L_cc z_c + D_c makes the
e-exprs constant-free.  sigmoid -> tanh (single ACT table: exp_and_others):
    1+sigmoid(kt) = (tanh(kt/2 + tb) + 3)/2
    out_o = (y_o/2 + m_o/2) * (th+3) + bo_o   (gate in M-rotated basis)
Custom DVE ops (registered at import, lowered into per-NEFF uop table) fuse
2-6 ALU stages per instruction.  PE absorbs some 3-stream affine combos as
diagonal-lhsT accumulating matmuls.  Engine assignment is cfg-tunable.
"""

import sys

if "/opt/trn_rl_repo" not in sys.path:
    sys.path.insert(0, "/opt/trn_rl_repo")

import numpy as np

import concourse.bass as bass
import concourse.bacc as bacc
import concourse.mybir as mybir
from concourse.tile import TileContext
from concourse import bass_utils

F32 = mybir.dt.float32
F16 = mybir.dt.float16
AF = mybir.ActivationFunctionType
OP = mybir.AluOpType

NCORES = 8
BPC = 2          # batches per core
ROWS = 64        # partition rows per batch
P = 128
FREE = 1024      # ROWS * FREE = H*W

# ---------------------------------------------------------------- custom ops
from concourse.dve_spec import (  # noqa: E402
    Spec, Src0, Src1, C0, C1, sq, lower, _has_src1,
)
import concourse.dve_ops as dve_ops  # noqa: E402
from concourse.dve_ops import DveOp  # noqa: E402
from concourse.dve_uop import DveOpSpec  # noqa: E402


def _register(name, body, reference):
    if name in dve_ops._SUB_OPCODE_FOR_NAME:
        return next(o for o in dve_ops.OPS if o.name == name)
    spec = Spec(body=body, reference=reference)
    op = DveOp(name, spec, subdim=False, uops_sha={})
    row = max(dve_ops._SUB_OPCODE_FOR_NAME.values()) + 1
    assert row < 0x20, "custom-DVE rows exhausted"
    dve_ops.OPS.append(op)
    dve_ops.CUSTOM_DVE_SPECS[name] = spec
    dve_ops._SUB_OPCODE_FOR_NAME[name] = row
    for ver in ("v3",):
        compiled = DveOpSpec(
            name=name, opcode=row, uops=lower(spec, ver=ver),
            rd1_en=_has_src1(spec),
        )
        op.uops_sha[ver] = compiled.sha(ver)
    return op


def _f32(x):
    return np.asarray(x, dtype=np.float32)


# A/C: sq(in0 + in1*s0) + sq(in1)*s1   (s1=1 -> e1^2+e2^2 ; s1=0 -> e0^2)
SQSQ = _register(
    "ANT_KF_SQSQ",
    sq(Src0 + Src1 * C0) + sq(Src1) * C1,
    lambda in0, in1, s0, s1, imm2: _f32(
        (in0.astype(np.float32) + in1 * s0) ** 2
        + (in1.astype(np.float32) ** 2) * s1),
)
# P/F/Y: in0 + in1*s0 + s1
AXPBC = _register(
    "ANT_KF_AXPBC",
    Src0 + Src1 * C0 + C1,
    lambda in0, in1, s0, s1, imm2: _f32(in0.astype(np.float32) + in1 * s0 + s1),
)
# H/E: in0*s0 + in1*s1
MULMUL = _register(
    "ANT_KF_MULMUL",
    Src0 * C0 + Src1 * C1,
    lambda in0, in1, s0, s1, imm2: _f32(
        in0.astype(np.float32) * s0 + in1.astype(np.float32) * s1),
)
# G: (in1*s0 +/- in0) + sq(in1)*s1    in0=krbf', in1=v(=k_lin-s)
KTP = _register(
    "ANT_KF_KTP",
    (Src1 * C0 + Src0) + sq(Src1) * C1,
    lambda in0, in1, s0, s1, imm2: _f32(
        in1.astype(np.float32) * s0 + in0 + (in1.astype(np.float32) ** 2) * s1),
)
KTM = _register(
    "ANT_KF_KTM",
    (Src1 * C0 - Src0) + sq(Src1) * C1,
    lambda in0, in1, s0, s1, imm2: _f32(
        in1.astype(np.float32) * s0 - in0 + (in1.astype(np.float32) ** 2) * s1),
)
# GATE2: (in0 + s0) * in1 + s1       in0=Y, in1=th3(=th+3), s0=m/2, s1=bo
GATE2 = _register(
    "ANT_KF_GATE2",
    (Src0 + C0) * Src1 + C1,
    lambda in0, in1, s0, s1, imm2: _f32(
        (in0.astype(np.float32) + s0) * in1 + s1),
)

# ------------------------------------------------------------- const layout
CN = ["D0", "D1", "D2",            # z'' biases (per-batch)
      "L00", "L11", "L22",         # z'' scales
      "LAM01", "LAM02", "LAM12",   # e-combine coefs
      "NEGG", "BETA0",             # exp scale/bias
      "U0", "U1", "U2",            # v = u.q coefs (per-batch)
      "KC0", "KC1",                # G coefs (per-batch via s-fold)
      "TB",                        # tanh bias (per-batch)
      "M00", "M01", "M02", "M10", "M11", "M12", "M20", "M21", "M22",  # M/2
      "MV0", "MV1", "MV2",         # m/2
      "BO0", "BO1", "BO2"]         # out bias
CI = {n: i for i, n in enumerate(CN)}
NCONST = len(CN)

_NC_CACHE: dict = {}

# engine assignment knobs
DEF_CFG = {
    "nchunk": 2,
    "z_eng": "act",          # z'' prep: 'act' (3 Identity) | 'dve' (3 ts)
    "kl_eng": "pe",          # v = u.q : 'pe' | 'dve' (2 customs) | 'act'
    "y_eng": ("pe", "act", "dve"),   # per-o ycheck combos
    "ymix_tt": "dve",        # tt engine for 'act' y variant: 'dve'|'pool'
    "d_eng": "pool",         # D = A + C add
    "gate_eng": ("dve", "dve", "dve"),  # 'dve' custom | 'mix'
    "out_q": ("sync", "pool", "sync"),  # DMA queue per out channel (sync|act|pool)
    "out16": True,
}


def _n_pe_diags(cfg):
    n = 0
    if cfg["kl_eng"] == "pe":
        n += 3
    for o in range(3):
        if cfg["y_eng"][o] == "pe":
            n += 3
    return n


def _build_nc(sw0_pos: bool, cfg=None):
    cfg = dict(DEF_CFG, **(cfg or {}))
    nchunk = cfg["nchunk"]
    cf = FREE // nchunk
    OUT_DT = F16 if cfg["out16"] else F32

    nc = bacc.Bacc("TRN2", target_bir_lowering=False)
    zc = nc.dram_tensor("zc", [P, 3 * FREE], F16, kind="ExternalInput")
    cb = nc.dram_tensor("cb", [P, NCONST], F32, kind="ExternalInput")
    ndiag = _n_pe_diags(cfg)
    if ndiag:
        dg = nc.dram_tensor("dg", [P, ndiag * P], F16, kind="ExternalInput")
    out = nc.dram_tensor("out_shard", [3, P, FREE], OUT_DT, kind="ExternalOutput")

    KT = KTP if sw0_pos else KTM

    def q_eng(which):
        return {"sync": nc.sync, "act": nc.scalar, "dve": nc.vector,
                "pool": nc.gpsimd}[which]

    with TileContext(nc) as tc:
        pools = [tc.tile_pool(name="cpool", bufs=1),
                 tc.tile_pool(name="work", bufs=1)]
        if ndiag:
            pools.append(tc.tile_pool(name="psum", bufs=1, space="PSUM"))
        with pools[0] as cpool, pools[1] as pool:
            psum = None
            if ndiag:
                psum_cm = pools[2]
                psum = psum_cm.__enter__()
            zt = cpool.tile([P, 3 * FREE], F16, name="zt")
            ct = cpool.tile([P, NCONST], F32, name="ct")
            nc.sync.dma_start(out=ct[:, :], in_=cb[:, :])
            if ndiag:
                dt_ = cpool.tile([P, ndiag * P], F16, name="dt")
                nc.scalar.dma_start(out=dt_[:, :], in_=dg[:, :])
            cw = FREE // nchunk
            for ci in range(nchunk):
                a = 3 * cw * ci
                nc.sync.dma_start(out=zt[:, a:a + 3 * cw],
                                  in_=zc[:, a:a + 3 * cw])

            def col(j):
                return ct[:, CI[j]:CI[j] + 1]

            diag_idx = {}
            if ndiag:
                k = 0
                if cfg["kl_eng"] == "pe":
                    for c in range(3):
                        diag_idx[("kl", c)] = k; k += 1
                for o in range(3):
                    if cfg["y_eng"][o] == "pe":
                        for c in range(3):
                            diag_idx[("y", o, c)] = k; k += 1

            for ci in range(nchunk):
                fs = ci * cf
                base = 3 * cw * ci
                z = [zt[:, base + c * cw:base + (c + 1) * cw] for c in range(3)]

                def t(tag, dt=F16):
                    return pool.tile([P, cf], dt, tag=f"{tag}_{ci}",
                                     name=f"{tag}_{ci}")

                # ---- z'' (shifted/scaled basis for dist path)
                zpp = []
                for c in range(3):
                    zc_t = t(f"zpp{c}")
                    if cfg["z_eng"] == "act":
                        nc.scalar.activation(zc_t[:, :], z[c][:, :], AF.Identity,
                                             bias=col(f"D{c}"),
                                             scale=col(f"L{c}{c}"))
                    else:
                        nc.vector.tensor_scalar(zc_t[:, :], z[c][:, :],
                                                col(f"L{c}{c}"), col(f"D{c}"),
                                                OP.mult, OP.add)
                    zpp.append(zc_t)

                # ---- dist = sq(e0)+sq(e1)+sq(e2)
                A = t("A")
                nc.vector._custom_dve(SQSQ, out=A[:, :], in0=zpp[1][:, :],
                                      in1=zpp[2][:, :], s0=col("LAM12"), s1=1.0)
                Pp = t("Pp")
                nc.vector._custom_dve(AXPBC, out=Pp[:, :], in0=zpp[0][:, :],
                                      in1=zpp[1][:, :], s0=col("LAM01"), s1=0.0)
                Cc = t("Cc")
                nc.vector._custom_dve(SQSQ, out=Cc[:, :], in0=Pp[:, :],
                                      in1=zpp[2][:, :], s0=col("LAM02"), s1=0.0)
                D = t("D")
                d_ng = {"pool": nc.gpsimd, "dve": nc.vector}[cfg["d_eng"]]
                d_ng.tensor_add(out=D[:, :], in0=A[:, :], in1=Cc[:, :])

                krbf = t("krbf")
                nc.scalar.activation(krbf[:, :], D[:, :], AF.Exp,
                                     bias=col("BETA0"), scale=col("NEGG"))

                # ---- v = u . q   (k_lin sans s; s folded into KC0/TB)
                if cfg["kl_eng"] == "pe":
                    v = psum.tile([P, cf], F32, tag=f"v_{ci}", name=f"v_{ci}")
                    for c in range(3):
                        k = diag_idx[("kl", c)]
                        nc.tensor.matmul(v[:, :], dt_[:, k * P:(k + 1) * P],
                                         z[c][:, :], start=(c == 0),
                                         stop=(c == 2))
                elif cfg["kl_eng"] == "dve":
                    E = t("E")
                    nc.vector._custom_dve(MULMUL, out=E[:, :], in0=z[0][:, :],
                                          in1=z[1][:, :], s0=col("U0"),
                                          s1=col("U1"))
                    v = t("v")
                    nc.vector._custom_dve(AXPBC, out=v[:, :], in0=E[:, :],
                                          in1=z[2][:, :], s0=col("U2"), s1=0.0)
                else:  # act
                    k0 = t("k0")
                    nc.scalar.activation(k0[:, :], z[0][:, :], AF.Identity,
                                         scale=col("U0"))
                    k1 = t("k1")
                    nc.scalar.activation(k1[:, :], z[1][:, :], AF.Identity,
                                         scale=col("U1"))
                    k2 = t("k2")
                    nc.scalar.activation(k2[:, :], z[2][:, :], AF.Identity,
                                         scale=col("U2"))
                    k01 = t("k01")
                    nc.vector.tensor_add(out=k01[:, :], in0=k0[:, :], in1=k1[:, :])
                    v = t("v")
                    nc.vector.tensor_add(out=v[:, :], in0=k01[:, :], in1=k2[:, :])

                # ---- kt/2 then th3 = tanh(kt/2 + tb) + 3
                G = t("G")
                nc.vector._custom_dve(KT, out=G[:, :], in0=krbf[:, :],
                                      in1=v[:, :], s0=col("KC0"), s1=col("KC1"))
                th = t("th")
                nc.scalar.activation(th[:, :], G[:, :], AF.Tanh, bias=col("TB"))
                th3 = t("th3")
                nc.vector.tensor_scalar(th3[:, :], th[:, :], 1.0, 3.0,
                                        OP.mult, OP.add)

                # ---- ycheck_o = (M_o . q)/2 ; out_o = (y+mv)*th3 + bo
                for o in range(3):
                    ye = cfg["y_eng"][o]
                    if ye == "pe":
                        Y = psum.tile([P, cf], F32, tag=f"Y{o}_{ci}",
                                      name=f"Y{o}_{ci}")
                        for c in range(3):
                            k = diag_idx[("y", o, c)]
                            nc.tensor.matmul(Y[:, :], dt_[:, k * P:(k + 1) * P],
                                             z[c][:, :], start=(c == 0),
                                             stop=(c == 2))
                    elif ye == "dve":
                        H = t(f"H{o}")
                        nc.vector._custom_dve(MULMUL, out=H[:, :],
                                              in0=z[0][:, :], in1=z[1][:, :],
                                              s0=col(f"M{o}0"), s1=col(f"M{o}1"))
                        Y = t(f"Y{o}")
                        nc.vector._custom_dve(AXPBC, out=Y[:, :], in0=H[:, :],
                                              in1=z[2][:, :], s0=col(f"M{o}2"),
                                              s1=0.0)
                    else:  # act
                        ya = t(f"ya{o}")
                        nc.scalar.activation(ya[:, :], z[0][:, :], AF.Identity,
                                             scale=col(f"M{o}0"))
                        yb = t(f"yb{o}")
                        nc.scalar.activation(yb[:, :], z[1][:, :], AF.Identity,
                                             scale=col(f"M{o}1"))
                        yc = t(f"yc{o}")
                        nc.scalar.activation(yc[:, :], z[2][:, :], AF.Identity,
                                             scale=col(f"M{o}2"))
                        tt_ng = {"dve": nc.vector, "pool": nc.gpsimd}[cfg["ymix_tt"]]
                        yab = t(f"yab{o}")
                        tt_ng.tensor_add(out=yab[:, :], in0=ya[:, :], in1=yb[:, :])
                        Y = t(f"Y{o}")
                        tt_ng.tensor_add(out=Y[:, :], in0=yab[:, :], in1=yc[:, :])

                    oo = t(f"oo{o}", OUT_DT)
                    if cfg["gate_eng"][o] == "dve":
                        nc.vector._custom_dve(GATE2, out=oo[:, :], in0=Y[:, :],
                                              in1=th3[:, :], s0=col(f"MV{o}"),
                                              s1=col(f"BO{o}"))
                    else:  # mix: Ym on ACT, mul+bias on DVE
                        Ym = t(f"Ym{o}")
                        nc.scalar.activation(Ym[:, :], Y[:, :], AF.Identity,
                                             bias=col(f"MV{o}"))
                        pr = t(f"pr{o}")
                        nc.vector.tensor_mul(out=pr[:, :], in0=Ym[:, :],
                                             in1=th3[:, :])
                        nc.vector.tensor_scalar(oo[:, :], pr[:, :], 1.0,
                                                col(f"BO{o}"), OP.mult, OP.add)
                    q_eng(cfg["out_q"][o]).dma_start(
                        out=out[o][:, fs:fs + cf], in_=oo[:, :])
            if ndiag:
                psum_cm.__exit__(None, None, None)
    nc.compile()
    return nc


def _get_nc(sw0_pos, sw2_pos=True, nchunk=None, use_gpsimd=None, cfg=None):
    # signature kept loosely compatible with the old test.py harness
    if isinstance(cfg, dict):
        c = dict(DEF_CFG, **cfg)
    else:
        c = dict(DEF_CFG)
    if isinstance(nchunk, int):
        c["nchunk"] = nchunk
    key = (bool(sw0_pos), tuple(sorted((k, str(v)) for k, v in c.items())))
    if key not in _NC_CACHE:
        _NC_CACHE[key] = _build_nc(bool(sw0_pos), c)
    return _NC_CACHE[key]


def _host_prep(inputs, cfg):
    d = {k: np.asarray(v, dtype=np.float64) for k, v in inputs.items()}
    z = np.asarray(inputs["z"], dtype=np.float32)
    B, C, H, W = z.shape
    Wz, bz = d["z_proj_w"], d["z_proj_b"]
    Wt, bt = d["text_proj_w"], d["text_proj_b"]
    Wo, bo = d["out_w"], d["out_b"]
    gamma = np.exp(d["log_gamma"])
    alpha, c_, w = d["alpha"], d["c"], d["w"]
    sumw = w.sum() + 1e-8
    w0p, w1p, w2p = w[0] / sumw, w[1] / sumw, w[2] / sumw

    t = d["text_vec"] @ Wt.T + bt                       # [B, HID]
    Gm = Wz.T @ Wz
    L = np.linalg.cholesky(Gm)                          # may raise
    delta = bz[None, :] - t                             # [B, HID]
    vv = delta @ Wz                                     # [B, 3]
    cdist = (delta ** 2).sum(1)
    r = np.linalg.solve(L, vv.T).T                      # [B, 3]
    rho = cdist - (r ** 2).sum(1)
    u = t @ Wz                                          # [B, 3]
    s = t @ bz                                          # [B]

    lam01 = L[1, 0] / L[1, 1]
    lam02 = L[2, 0] / L[2, 2]
    lam12 = L[2, 1] / L[2, 2]
    # solve U D = r with U[i,c] = L[c,i]/L_cc (c>=i)
    Dsh = np.zeros((B, 3))
    Dsh[:, 2] = r[:, 2]
    Dsh[:, 1] = r[:, 1] - lam12 * Dsh[:, 2]
    Dsh[:, 0] = r[:, 0] - lam01 * Dsh[:, 1] - lam02 * Dsh[:, 2]

    sw0 = bool(w0p >= 0.0)
    sw2v = 1.0 if w2p >= 0.0 else -1.0
    a = alpha * np.sqrt(abs(w2p))
    b = c_ * np.sqrt(abs(w2p))
    if w0p == 0.0:
        beta0 = np.full(B, -1e30)
    else:
        beta0 = -gamma * rho + np.log(abs(w0p) / 2.0)
    kc0 = 0.5 * (w1p + 2.0 * a * b * sw2v)
    kc1 = 0.5 * sw2v * a * a
    # fold s (k_lin = v + s):  kt/2 += kc0*s + kc1*s^2 ; coef of v += 2*kc1*s
    kc0_b = kc0 + 2.0 * kc1 * s
    tb = kc0 * s + kc1 * s * s + 0.5 * sw2v * b * b

    M = Wo @ Wz
    m = Wo @ bz

    cbv = np.zeros((B, NCONST), dtype=np.float64)
    for c in range(3):
        cbv[:, CI[f"D{c}"]] = Dsh[:, c]
        cbv[:, CI[f"L{c}{c}"]] = L[c, c]
        cbv[:, CI[f"U{c}"]] = u[:, c]
        cbv[:, CI[f"MV{c}"]] = m[c] / 2.0
        cbv[:, CI[f"BO{c}"]] = bo[c]
        for cc in range(3):
            cbv[:, CI[f"M{c}{cc}"]] = M[c, cc] / 2.0
    cbv[:, CI["LAM01"]] = lam01
    cbv[:, CI["LAM02"]] = lam02
    cbv[:, CI["LAM12"]] = lam12
    cbv[:, CI["NEGG"]] = -gamma
    cbv[:, CI["BETA0"]] = beta0
    cbv[:, CI["KC0"]] = kc0_b
    cbv[:, CI["KC1"]] = kc1
    cbv[:, CI["TB"]] = tb
    cbv = cbv.astype(np.float32)

    # PE diagonal tiles
    ndiag = _n_pe_diags(cfg)
    diag_specs = []      # per diag: [B] values
    if cfg["kl_eng"] == "pe":
        for c in range(3):
            diag_specs.append(u[:, c])
    for o in range(3):
        if cfg["y_eng"][o] == "pe":
            for c in range(3):
                diag_specs.append(np.full(B, M[o, c] / 2.0))
    assert len(diag_specs) == ndiag

    in_maps = []
    zh = z.astype(np.float16)
    for core in range(NCORES):
        packed = np.empty((P, 3 * FREE), dtype=np.float16)
        cs = np.empty((P, NCONST), dtype=np.float32)
        cw = FREE // cfg["nchunk"]
        for j in range(BPC):
            bidx = core * BPC + j
            zr = zh[bidx].reshape(3, ROWS, FREE)
            rows = slice(j * ROWS, (j + 1) * ROWS)
            for ci in range(cfg["nchunk"]):
                base = 3 * cw * ci
                for c in range(3):
                    packed[rows, base + c * cw:base + (c + 1) * cw] = \
                        zr[c, :, ci * cw:(ci + 1) * cw]
            cs[rows, :] = cbv[bidx]
        im = {"zc": packed, "cb": cs}
        if ndiag:
            dgt = np.zeros((P, ndiag * P), dtype=np.float16)
            for k, vals in enumerate(diag_specs):
                for j in range(BPC):
                    bidx = core * BPC + j
                    for pp in range(j * ROWS, (j + 1) * ROWS):
                        dgt[pp, k * P + pp] = np.float16(vals[bidx])
            im["dg"] = dgt
        in_maps.append(im)
    return in_maps, sw0, (B, C, H, W)


def _numpy_fallback(inputs):
    d = {k: np.asarray(v, dtype=np.float64) for k, v in inputs.items()}
    z, Wz, bz = d["z"], d["z_proj_w"], d["z_proj_b"]
    t = d["text_vec"] @ d["text_proj_w"].T + d["text_proj_b"]
    zm = np.einsum("bchw,oc->bohw", z, Wz) + bz[None, :, None, None]
    gamma = np.exp(d["log_gamma"])
    diff = zm - t[:, :, None, None]
    dist = (diff * diff).sum(1)
    klin = np.einsum("bchw,bc->bhw", zm, t)
    krbf = np.exp(-gamma * dist)
    kpoly = (d["alpha"] * klin + d["c"]) ** 2
    w = d["w"]
    k = (w[0] * krbf + w[1] * klin + w[2] * kpoly) / (w.sum() + 1e-8)
    zf = zm * (1.0 + 1.0 / (1.0 + np.exp(-k[:, None])))
    out = np.einsum("bchw,oc->bohw", zf, d["out_w"]) + d["out_b"][None, :, None, None]
    return out.astype(np.float32)


BEST_CFG: dict = dict(DEF_CFG)
BEST_NCHUNK = BEST_CFG["nchunk"]
BEST_GPSIMD = False


def run(inputs, trace=False, nchunk=None, use_gpsimd=None, cfg=None):
    c = dict(BEST_CFG if cfg is None else dict(DEF_CFG, **cfg))
    if isinstance(nchunk, int):
        c["nchunk"] = nchunk
    try:
        in_maps, sw0, (B, C, H, W) = _host_prep(inputs, c)
    except np.linalg.LinAlgError:
        return _numpy_fallback(inputs), None
    nc = _get_nc(sw0, True, c["nchunk"], None, c)
    res = bass_utils.run_bass_kernel_spmd(
        nc, in_maps, core_ids=list(range(NCORES)), trace=trace)
    out = np.empty((B, C, H, W), dtype=np.float32)
    for core in range(NCORES):
        o = np.asarray(res.results[core]["out_shard"], dtype=np.float32)
        for j in range(BPC):
            b = core * BPC + j
            out[b] = o[:, j * ROWS:(j + 1) * ROWS, :].reshape(C, H, W)
    return out, res


def kernel(**inputs):
    out, _ = run(inputs, trace=False)
    return out
